# revision 9
# baseline (speedup 1.0000x reference)
"""Segment-softmax (GAT attention stage 4) Trainium2 kernel, 8 NeuronCores.

alpha_i = exp(e_i) / (sum_{j: tgt_j = tgt_i} exp(e_j) + 1e-16)

Strategy (node-parallel sharding, degree-sorted variable-width packing):
  - The host stable-sorts edges by target node (a pure data-layout
    permutation, inverted after the device run) and shards NODES across the
    8 cores (12500 nodes each) -> each core owns complete segments, so no
    cross-core reduction is needed.
  - Within each core, nodes are ordered by degree (ascending); blocks of
    128 consecutive nodes map to the 128 SBUF partitions.  Blocks are
    grouped into chunks, and every node row in a chunk is padded to the
    chunk's max degree W (rounded up to 8).  Because degrees are sorted,
    total padding is only a few % over the raw edge count.  The chunk plan
    (nb, W) is derived from the input's degree histogram at first call and
    shared across all cores (max over cores per block index).
  - The host embeds I/8192 (exact in fp16) at column 0; the PE partial sums
    then come out pre-scaled by 1/8192, so the reciprocal directly yields
    8192/S, which keeps the smallest alpha*8192 in fp16 normal range with
    no separate scale/clamp pass.
  - Device work per chunk, spread across four engines:
      ACT:  X = exp(E)                        (fp16, padding -100 -> 0)
      PE:   PS[p, n*8+c] += X[p, n, 8*j+c]/8192   (identity-matmul partial
            sums accumulated in one PSUM bank; W/8 matmuls on the idle PE)
      DVE:  S[p,n] = reduce_add(PS view)      (fp32, only nb*8 elements)
      DVE:  R16d[p,n,t] = recip_approx_fast(S[p,n]) -> f16, 8-dup
            (one custom-DVE op: broadcast read + f16 write fuse the
            reciprocal, the f16 cast and the 8x duplication)
      DVE:  A = X * R16d                      (f16 2x mode: the stride-0
            broadcast hides in a middle dim of a [P,nb,W/8,8] view while
            the innermost dim reads unit-stride 8-duplicated runs)
  - Chunk processing is big-first after the small opener: chunk 0 is small
    so the first exp starts during the DMA latency, then the largest chunks
    run while the DVE backlog can still hide behind the exp stream, and the
    tail chunks are tiny so the final store chain drains quickly.  The
    final store issues on the ACT HWDGE ring (idle after the last exp) so
    it does not queue behind the penultimate store on the Sync ring.
  - Host divides by 8192 (exact) while scattering back to original order.
  All arithmetic (exp, segment sums, reciprocal, normalize) runs on device;
  the host only sorts/pads/permutes layouts.
"""
import sys

sys.path.insert(0, "/opt/trn_rl_repo")

import numpy as np
import concourse.bacc as bacc
import concourse.mybir as mybir
import concourse.tile as tile
from concourse import bass_utils
from concourse.dve_ops import RECIP_APPROX_FAST_CONSTS, RECIPROCAL_APPROX_FAST

P = 128
N_CORES = 8
NUM_EDGES = 6_400_000
NUM_NODES = 100_000
NPC = NUM_NODES // N_CORES          # 12500 nodes per core
NBLK = (NPC + P - 1) // P           # 98 node blocks per core
SCALE = 8192.0                      # keeps alpha*SCALE in fp16 normal range
FIRST_FD = 512                      # small first chunk: start compute early
TARGET_FD = 1792                    # mid chunks
TAIL_FDS = (640, 320)               # taper the tail: drain quickly
CAP_FD = 2304
MAXNB = 64
MM_W = 8                            # identity-matmul sub-slice width
DUP = 8                             # duplication factor of the R row

f16, f32 = mybir.dt.float16, mybir.dt.float32
_cache = {}


def make_plan(counts):
    """Chunk plan [(nb, W), ...] covering the NBLK degree-sorted blocks."""
    deg_sorted = np.sort(counts.reshape(N_CORES, NPC), axis=1)
    pad = NBLK * P - NPC
    deg_sorted = np.pad(deg_sorted, ((0, 0), (0, pad)))
    blockmax = deg_sorted.reshape(N_CORES, NBLK, P).max(axis=2).max(axis=0)
    blockmax = np.maximum(blockmax, 1)
    wof = ((blockmax + 7) // 8) * 8          # per-block width if chunk ended
    plan = []
    b = 0
    while b < NBLK:
        ci = len(plan)
        tgt = FIRST_FD if ci == 0 else (1024 if ci == 1 else TARGET_FD)
        nb = 1
        while b + nb < NBLK and (nb + 1) * wof[b + nb] <= tgt:
            nb += 1
        plan.append((int(nb), int(wof[b + nb - 1])))
        b += nb
    # carve small tail chunks so the final stores are short
    for tail_fd in TAIL_FDS:
        if len(plan) > 1 and plan[-1][0] * plan[-1][1] > tail_fd:
            nb, W = plan.pop()
            nb_tail = max(1, tail_fd // W)
            if nb > nb_tail:
                plan.append((nb - nb_tail, W))
            plan.append((min(nb, nb_tail), W))
    return tuple(plan)


def plan_layout(plan):
    """Per-block (colbase, width) arrays and chunk offsets.

    Column layout: [identity (P cols) | chunk 0 | chunk 1 | ...]."""
    W_blk = np.empty(NBLK, dtype=np.int64)
    base_blk = np.empty(NBLK, dtype=np.int64)
    chunk_off = []
    o = P
    b = 0
    for nb, W in plan:
        chunk_off.append(o)
        for i in range(nb):
            W_blk[b + i] = W
            base_blk[b + i] = o + i * W
        o += nb * W
        b += nb
    assert b == NBLK
    return W_blk, base_blk, chunk_off, o


PSUM_BANK = 512                     # one PSUM bank: 512 fp32 per partition


def process_order(plan):
    """Chunk 0 first (small, hides DMA latency), then biggest-first so the
    DVE backlog drains during the exp stream and the tail is tiny."""
    rest = sorted(range(1, len(plan)),
                  key=lambda ci: -plan[ci][0] * plan[ci][1])
    return [0] + rest


def _build(plan):
    W_blk, base_blk, chunk_off, FD = plan_layout(plan)
    nc = bacc.Bacc("TRN2", target_bir_lowering=False, debug=False,
                   enable_asserts=False)
    d_E = nc.dram_tensor("E", [P, FD], f16, kind="ExternalInput")
    d_A = nc.dram_tensor("alpha", [P, FD], f16, kind="ExternalOutput")
    OP = mybir.AluOpType
    Exp = mybir.ActivationFunctionType.Exp
    order = process_order(plan)
    rc = RECIP_APPROX_FAST_CONSTS

    with tile.TileContext(nc) as tc:
        with (
            tc.tile_pool(name="const", bufs=1) as cpool,
            tc.tile_pool(name="io", bufs=6) as iopool,
            tc.tile_pool(name="sm", bufs=4) as spool,
            tc.tile_pool(name="ps", bufs=3, space="PSUM") as ppool,
        ):
            # first DMA: the scaled identity + the small chunk 0, so exp and
            # the first PE matmuls start as soon as one small load lands
            c0_end = chunk_off[0] + plan[0][0] * plan[0][1]
            E0 = cpool.tile([P, c0_end], f16)
            nc.sync.dma_start(out=E0[:], in_=d_E[:, 0:c0_end])
            ident = E0[:, 0:P]
            for ci in order:
                nb, W = plan[ci]
                o_lo = chunk_off[ci]
                fdc = nb * W
                assert nb <= MAXNB and fdc <= CAP_FD
                assert W % MM_W == 0 and nb * MM_W <= PSUM_BANK
                if ci == 0:
                    E16v = E0[:, o_lo:o_lo + fdc]
                else:
                    E16 = iopool.tile([P, CAP_FD], f16, tag="E16")
                    nc.sync.dma_start(out=E16[:, 0:fdc],
                                      in_=d_E[:, o_lo:o_lo + fdc])
                    E16v = E16[:, 0:fdc]
                X16 = iopool.tile([P, CAP_FD], f16, tag="X16")
                nc.scalar.activation(X16[:, 0:fdc], E16v, Exp)
                # segment partial sums on the (otherwise idle) PE: accumulate
                # identity-matmuls of MM_W-wide sub-slices into PSUM, so the
                # vector engine only reduces nb*MM_W elements
                v = X16[:, 0:fdc].rearrange("p (n d) -> p n d", d=W)
                S = spool.tile([P, MAXNB], f32, tag="S")
                PS = ppool.tile([P, PSUM_BANK], f32, space="PSUM", tag="PS")
                nmm = W // MM_W
                for j in range(nmm):
                    nc.tensor.matmul(out=PS[:, 0:nb * MM_W],
                                     lhsT=ident[:],
                                     rhs=v[:, :, MM_W * j:MM_W * (j + 1)],
                                     start=(j == 0), stop=(j == nmm - 1))
                nc.vector.tensor_reduce(
                    out=S[:, 0:nb],
                    in_=PS[:, 0:nb * MM_W].rearrange("p (n d) -> p n d",
                                                     d=MM_W),
                    axis=mybir.AxisListType.X, op=OP.add)
                # one fused custom-DVE op: R16d[p, n, t] = f16(1/S[p, n])
                # (= 8192/sum since the identity is pre-scaled), written as
                # 8-duplicated unit-stride runs for the broadcast multiply
                R16d = spool.tile([P, DUP * MAXNB], f16, tag="R16d")
                nc.vector._custom_dve(
                    RECIPROCAL_APPROX_FAST,
                    out=R16d[:, 0:DUP * nb].rearrange("p (n t) -> p n t",
                                                      t=DUP),
                    in0=S[:, 0:nb].unsqueeze(2).broadcast_to([P, nb, DUP]),
                    s0=rc["s0"], s1=rc["s1"], imm2=rc["imm2"])
                A16 = iopool.tile([P, CAP_FD], f16, tag="A16")
                gW = W // DUP
                xv = X16[:, 0:fdc].rearrange("p (n g t) -> p n g t",
                                             g=gW, t=DUP)
                av = A16[:, 0:fdc].rearrange("p (n g t) -> p n g t",
                                             g=gW, t=DUP)
                rb = R16d[:, 0:DUP * nb].rearrange(
                    "p (n t) -> p n t", t=DUP).unsqueeze(2).broadcast_to(
                    [P, nb, gW, DUP])
                nc.vector.tensor_tensor(out=av, in0=xv, in1=rb, op=OP.mult)
                # the final store issues on the ACT HWDGE ring (idle once the
                # last exp is done) so it drains in parallel with the
                # penultimate store on the Sync ring instead of behind it
                store_eng = (nc.scalar if ci == order[-1] else nc.sync)
                store_eng.dma_start(out=d_A[:, o_lo:o_lo + fdc],
                                    in_=A16[:, 0:fdc])
    nc.compile()
    return nc


def _get_neff(plan):
    if plan not in _cache:
        _cache[plan] = _build(plan)
    return _cache[plan]


def prep_inputs(e, edge_index):
    """Sort edges by target node, degree-sort nodes, chunk-width padding."""
    e = np.asarray(e, dtype=np.float32).reshape(-1)
    t = np.asarray(edge_index)[1].astype(np.int64)
    counts = np.bincount(t, minlength=NUM_NODES)
    plan = make_plan(counts)
    W_blk, base_blk, chunk_off, FD = plan_layout(plan)
    # node -> rank within its core under ascending-degree order
    order = np.argsort(counts.reshape(N_CORES, NPC), axis=1, kind="stable")
    m_of = np.empty((N_CORES, NPC), dtype=np.int64)
    ar = np.arange(NPC, dtype=np.int64)
    for c in range(N_CORES):
        m_of[c, order[c]] = ar
    m = m_of.reshape(-1)                    # global node -> rank in core
    p_of = m % P
    colbase = base_blk[m // P]              # start column per node
    # per-edge destination in the padded layout
    perm = np.argsort(t, kind="stable")
    t_s = t[perm]
    starts = np.zeros(NUM_NODES + 1, dtype=np.int64)
    np.cumsum(counts, out=starts[1:])
    rank = np.arange(NUM_EDGES, dtype=np.int64) - starts[t_s]
    c_e = t_s // NPC
    flat = (c_e * P + p_of[t_s]) * FD + colbase[t_s] + rank
    E = np.full(N_CORES * P * FD, -100.0, dtype=np.float16)
    E[flat] = e[perm].astype(np.float16)
    E = E.reshape(N_CORES, P, FD)
    # scaled identity at column 0: PE sums come out as S/8192, so the
    # reciprocal directly yields 8192/S (1/8192 is exact in fp16)
    E[:, :, 0:P] = (np.eye(P, dtype=np.float16) / np.float16(SCALE))[None]
    return E, flat, perm, plan


def make_in_maps(E):
    return [{"E": E[c]} for c in range(N_CORES)]


def kernel(e, edge_index, num_nodes):
    assert int(num_nodes) == NUM_NODES
    E, flat, perm, plan = prep_inputs(e, edge_index)
    nc = _get_neff(plan)
    in_maps = make_in_maps(E)
    res = bass_utils.run_bass_kernel_spmd(nc, in_maps,
                                          core_ids=list(range(N_CORES)))
    A = np.stack([np.asarray(res.results[c]["alpha"])
                  for c in range(N_CORES)])
    alpha_sorted = A.reshape(-1)[flat].astype(np.float32) * np.float32(1.0 / SCALE)
    out = np.empty(NUM_EDGES, dtype=np.float32)
    out[perm] = alpha_sorted
    return out


# revision 10
# speedup vs baseline: 1.1081x; 1.1081x over previous
"""Segment-softmax (GAT attention stage 4) Trainium2 kernel, 8 NeuronCores.

alpha_i = exp(e_i) / (sum_{j: tgt_j = tgt_i} exp(e_j) + 1e-16)

Strategy (node-parallel sharding, degree-sorted variable-width packing):
  - The host stable-sorts edges by target node (a pure data-layout
    permutation, inverted after the device run) and shards NODES across the
    8 cores (12500 nodes each) -> each core owns complete segments, so no
    cross-core reduction is needed.
  - Within each core, nodes are ordered by degree (ascending); blocks of
    128 consecutive nodes map to the 128 SBUF partitions.  Blocks are
    grouped into chunks, and every node row in a chunk is padded to the
    chunk's max degree W (rounded up to 8).  Because degrees are sorted,
    total padding is only a few % over the raw edge count.  The chunk plan
    (nb, W) is derived from the input's degree histogram at first call and
    shared across all cores (max over cores per block index).
  - The host embeds I/8192 (exact in fp16) at column 0; the PE partial sums
    then come out pre-scaled by 1/8192, so the reciprocal directly yields
    8192/S, which keeps the smallest alpha*8192 in fp16 normal range with
    no separate scale/clamp pass.
  - Device work per chunk, spread across four engines:
      ACT:  X = exp(E)                        (fp16, padding -100 -> 0)
      PE:   PS[p, n*8+c] += X[p, n, 8*j+c]/8192   (identity-matmul partial
            sums accumulated in one PSUM bank; W/8 matmuls on the idle PE)
      DVE:  S[p,n] = reduce_add(PS view)      (fp32, only nb*8 elements)
      DVE:  R16d[p,n,t] = recip_approx_fast(S[p,n]) -> f16, 8-dup
            (one custom-DVE op: broadcast read + f16 write fuse the
            reciprocal, the f16 cast and the 8x duplication)
      DVE:  A = X * R16d                      (f16 2x mode: the stride-0
            broadcast hides in a middle dim of a [P,nb,W/8,8] view while
            the innermost dim reads unit-stride 8-duplicated runs)
  - Chunk processing is big-first after the small opener: chunk 0 is small
    so the first exp starts during the DMA latency, then the largest chunks
    run while the DVE backlog can still hide behind the exp stream, and the
    tail chunks are tiny so the final store chain drains quickly.  The
    final store issues on the ACT HWDGE ring (idle after the last exp) so
    it does not queue behind the penultimate store on the Sync ring.
  - Host divides by 8192 (exact) while scattering back to original order.
  All arithmetic (exp, segment sums, reciprocal, normalize) runs on device;
  the host only sorts/pads/permutes layouts.
"""
import sys

sys.path.insert(0, "/opt/trn_rl_repo")

import numpy as np
import concourse.bacc as bacc
import concourse.mybir as mybir
import concourse.tile as tile
from concourse import bass_utils
from concourse.dve_ops import RECIP_APPROX_FAST_CONSTS, RECIPROCAL_APPROX_FAST

P = 128
N_CORES = 8
NUM_EDGES = 6_400_000
NUM_NODES = 100_000
NPC = NUM_NODES // N_CORES          # 12500 nodes per core
NBLK = (NPC + P - 1) // P           # 98 node blocks per core
SCALE = 8192.0                      # keeps alpha*SCALE in fp16 normal range
FIRST_FD = 512                      # small first chunk: start compute early
TARGET_FD = 1792                    # mid chunks
TAIL_FDS = (640, 320)               # taper the tail: drain quickly
CAP_FD = 2304
MAXNB = 64
MM_W = 8                            # identity-matmul sub-slice width
DUP = 8                             # duplication factor of the R row

f16, f32 = mybir.dt.float16, mybir.dt.float32
_cache = {}


def make_plan(counts):
    """Chunk plan [(nb, W), ...] covering the NBLK degree-sorted blocks."""
    deg_sorted = np.sort(counts.reshape(N_CORES, NPC), axis=1)
    pad = NBLK * P - NPC
    deg_sorted = np.pad(deg_sorted, ((0, 0), (0, pad)))
    blockmax = deg_sorted.reshape(N_CORES, NBLK, P).max(axis=2).max(axis=0)
    blockmax = np.maximum(blockmax, 1)
    wof = ((blockmax + 7) // 8) * 8          # per-block width if chunk ended
    plan = []
    b = 0
    while b < NBLK:
        ci = len(plan)
        tgt = FIRST_FD if ci == 0 else (1024 if ci == 1 else TARGET_FD)
        nb = 1
        while b + nb < NBLK and (nb + 1) * wof[b + nb] <= tgt:
            nb += 1
        plan.append((int(nb), int(wof[b + nb - 1])))
        b += nb
    # carve small tail chunks so the final stores are short
    for tail_fd in TAIL_FDS:
        if len(plan) > 1 and plan[-1][0] * plan[-1][1] > tail_fd:
            nb, W = plan.pop()
            nb_tail = max(1, tail_fd // W)
            if nb > nb_tail:
                plan.append((nb - nb_tail, W))
            plan.append((min(nb, nb_tail), W))
    return tuple(plan)


def plan_layout(plan):
    """Per-block (colbase, width) arrays and chunk offsets.

    Column layout: [identity (P cols) | chunk 0 | chunk 1 | ...]."""
    W_blk = np.empty(NBLK, dtype=np.int64)
    base_blk = np.empty(NBLK, dtype=np.int64)
    chunk_off = []
    o = P
    b = 0
    for nb, W in plan:
        chunk_off.append(o)
        for i in range(nb):
            W_blk[b + i] = W
            base_blk[b + i] = o + i * W
        o += nb * W
        b += nb
    assert b == NBLK
    return W_blk, base_blk, chunk_off, o


PSUM_BANK = 512                     # one PSUM bank: 512 fp32 per partition


def process_order(plan):
    """Chunk 0 first (small, hides DMA latency), then biggest-first so the
    DVE backlog drains during the exp stream and the tail is tiny."""
    rest = sorted(range(1, len(plan)),
                  key=lambda ci: -plan[ci][0] * plan[ci][1])
    return [0] + rest


def _build(plan):
    W_blk, base_blk, chunk_off, FD = plan_layout(plan)
    nc = bacc.Bacc("TRN2", target_bir_lowering=False, debug=False,
                   enable_asserts=False)
    d_E = nc.dram_tensor("E", [P, FD], f16, kind="ExternalInput")
    d_A = nc.dram_tensor("alpha", [P, FD], f16, kind="ExternalOutput")
    OP = mybir.AluOpType
    Exp = mybir.ActivationFunctionType.Exp
    order = process_order(plan)
    rc = RECIP_APPROX_FAST_CONSTS

    with tile.TileContext(nc) as tc:
        with (
            tc.tile_pool(name="const", bufs=1) as cpool,
            tc.tile_pool(name="io", bufs=6) as iopool,
            tc.tile_pool(name="sm", bufs=4) as spool,
            tc.tile_pool(name="ps", bufs=3, space="PSUM") as ppool,
        ):
            # first DMA: the scaled identity + the small chunk 0, so exp and
            # the first PE matmuls start as soon as one small load lands
            c0_end = chunk_off[0] + plan[0][0] * plan[0][1]
            E0 = cpool.tile([P, c0_end], f16)
            nc.sync.dma_start(out=E0[:], in_=d_E[:, 0:c0_end])
            ident = E0[:, 0:P]
            for ci in order:
                nb, W = plan[ci]
                o_lo = chunk_off[ci]
                fdc = nb * W
                assert nb <= MAXNB and fdc <= CAP_FD
                assert W % MM_W == 0 and nb * MM_W <= PSUM_BANK
                if ci == 0:
                    E16v = E0[:, o_lo:o_lo + fdc]
                else:
                    E16 = iopool.tile([P, CAP_FD], f16, tag="E16")
                    nc.sync.dma_start(out=E16[:, 0:fdc],
                                      in_=d_E[:, o_lo:o_lo + fdc])
                    E16v = E16[:, 0:fdc]
                X16 = iopool.tile([P, CAP_FD], f16, tag="X16")
                A16 = iopool.tile([P, CAP_FD], f16, tag="A16")
                # split big chunks' compute into two nb-halves: half 1's
                # exp/PE/reduce/recip/multiply/store all run while half 2 is
                # still in exp, so stores start ~2us earlier per chunk and
                # the DMA wire never goes idle between the load stream and
                # the store stream
                gW = W // DUP
                subs = ([0, (nb + 1) // 2, nb] if fdc >= 1024 else [0, nb])
                for lo, hi in zip(subs, subs[1:]):
                    nbs = hi - lo
                    c_lo = lo * W
                    c_hi = hi * W
                    nc.scalar.activation(X16[:, c_lo:c_hi],
                                         E16v[:, c_lo:c_hi], Exp)
                    # segment partial sums on the (otherwise idle) PE:
                    # accumulate identity-matmuls of MM_W-wide sub-slices
                    # into PSUM, so the vector engine only reduces nb*MM_W
                    v = X16[:, c_lo:c_hi].rearrange("p (n d) -> p n d", d=W)
                    S = spool.tile([P, MAXNB], f32, tag="S")
                    PS = ppool.tile([P, PSUM_BANK], f32, space="PSUM",
                                    tag="PS")
                    nmm = W // MM_W
                    for j in range(nmm):
                        nc.tensor.matmul(out=PS[:, 0:nbs * MM_W],
                                         lhsT=ident[:],
                                         rhs=v[:, :, MM_W * j:MM_W * (j + 1)],
                                         start=(j == 0), stop=(j == nmm - 1))
                    nc.vector.tensor_reduce(
                        out=S[:, 0:nbs],
                        in_=PS[:, 0:nbs * MM_W].rearrange("p (n d) -> p n d",
                                                          d=MM_W),
                        axis=mybir.AxisListType.X, op=OP.add)
                    # one fused custom-DVE op: R16d[p, n, t] = f16(1/S[p, n])
                    # (= 8192/sum since the identity is pre-scaled), written
                    # as 8-duplicated unit-stride runs for the broadcast
                    # multiply
                    R16d = spool.tile([P, DUP * MAXNB], f16, tag="R16d")
                    nc.vector._custom_dve(
                        RECIPROCAL_APPROX_FAST,
                        out=R16d[:, 0:DUP * nbs].rearrange(
                            "p (n t) -> p n t", t=DUP),
                        in0=S[:, 0:nbs].unsqueeze(2).broadcast_to(
                            [P, nbs, DUP]),
                        s0=rc["s0"], s1=rc["s1"], imm2=rc["imm2"])
                    xv = X16[:, c_lo:c_hi].rearrange("p (n g t) -> p n g t",
                                                     g=gW, t=DUP)
                    av = A16[:, c_lo:c_hi].rearrange("p (n g t) -> p n g t",
                                                     g=gW, t=DUP)
                    rb = R16d[:, 0:DUP * nbs].rearrange(
                        "p (n t) -> p n t", t=DUP).unsqueeze(2).broadcast_to(
                        [P, nbs, gW, DUP])
                    nc.vector.tensor_tensor(out=av, in0=xv, in1=rb,
                                            op=OP.mult)
                    # the final store issues on the ACT HWDGE ring (idle once
                    # the last exp is done) so it drains in parallel with the
                    # penultimate store on the Sync ring instead of behind it
                    store_eng = (nc.scalar
                                 if ci == order[-1] and hi == nb else nc.sync)
                    store_eng.dma_start(
                        out=d_A[:, o_lo + c_lo:o_lo + c_hi],
                        in_=A16[:, c_lo:c_hi])
    nc.compile()
    return nc


def _get_neff(plan):
    if plan not in _cache:
        _cache[plan] = _build(plan)
    return _cache[plan]


def prep_inputs(e, edge_index):
    """Sort edges by target node, degree-sort nodes, chunk-width padding."""
    e = np.asarray(e, dtype=np.float32).reshape(-1)
    t = np.asarray(edge_index)[1].astype(np.int64)
    counts = np.bincount(t, minlength=NUM_NODES)
    plan = make_plan(counts)
    W_blk, base_blk, chunk_off, FD = plan_layout(plan)
    # node -> rank within its core under ascending-degree order
    order = np.argsort(counts.reshape(N_CORES, NPC), axis=1, kind="stable")
    m_of = np.empty((N_CORES, NPC), dtype=np.int64)
    ar = np.arange(NPC, dtype=np.int64)
    for c in range(N_CORES):
        m_of[c, order[c]] = ar
    m = m_of.reshape(-1)                    # global node -> rank in core
    p_of = m % P
    colbase = base_blk[m // P]              # start column per node
    # per-edge destination in the padded layout
    perm = np.argsort(t, kind="stable")
    t_s = t[perm]
    starts = np.zeros(NUM_NODES + 1, dtype=np.int64)
    np.cumsum(counts, out=starts[1:])
    rank = np.arange(NUM_EDGES, dtype=np.int64) - starts[t_s]
    c_e = t_s // NPC
    flat = (c_e * P + p_of[t_s]) * FD + colbase[t_s] + rank
    E = np.full(N_CORES * P * FD, -100.0, dtype=np.float16)
    E[flat] = e[perm].astype(np.float16)
    E = E.reshape(N_CORES, P, FD)
    # scaled identity at column 0: PE sums come out as S/8192, so the
    # reciprocal directly yields 8192/S (1/8192 is exact in fp16)
    E[:, :, 0:P] = (np.eye(P, dtype=np.float16) / np.float16(SCALE))[None]
    return E, flat, perm, plan


def make_in_maps(E):
    return [{"E": E[c]} for c in range(N_CORES)]


def kernel(e, edge_index, num_nodes):
    assert int(num_nodes) == NUM_NODES
    E, flat, perm, plan = prep_inputs(e, edge_index)
    nc = _get_neff(plan)
    in_maps = make_in_maps(E)
    res = bass_utils.run_bass_kernel_spmd(nc, in_maps,
                                          core_ids=list(range(N_CORES)))
    A = np.stack([np.asarray(res.results[c]["alpha"])
                  for c in range(N_CORES)])
    alpha_sorted = A.reshape(-1)[flat].astype(np.float32) * np.float32(1.0 / SCALE)
    out = np.empty(NUM_EDGES, dtype=np.float32)
    out[perm] = alpha_sorted
    return out


# revision 12
# speedup vs baseline: 1.1331x; 1.0226x over previous
"""Segment-softmax (GAT attention stage 4) Trainium2 kernel, 8 NeuronCores.

alpha_i = exp(e_i) / (sum_{j: tgt_j = tgt_i} exp(e_j) + 1e-16)

Strategy (node-parallel sharding, degree-sorted variable-width packing):
  - The host stable-sorts edges by target node (a pure data-layout
    permutation, inverted after the device run) and shards NODES across the
    8 cores (12500 nodes each) -> each core owns complete segments, so no
    cross-core reduction is needed.
  - Within each core, nodes are ordered by degree (ascending); blocks of
    128 consecutive nodes map to the 128 SBUF partitions.  Blocks are
    grouped into chunks, and every node row in a chunk is padded to the
    chunk's max degree W (rounded up to 8).  Because degrees are sorted,
    total padding is only a few % over the raw edge count.  The chunk plan
    (nb, W) is derived from the input's degree histogram at first call and
    shared across all cores (max over cores per block index).
  - The host embeds I/8192 (exact in fp16) at column 0; the PE partial sums
    then come out pre-scaled by 1/8192, so the reciprocal directly yields
    8192/S, which keeps the smallest alpha*8192 in fp16 normal range with
    no separate scale/clamp pass.
  - Device work per chunk, spread across four engines:
      ACT:  X = exp(E)                        (fp16, padding -100 -> 0)
      PE:   PS[p, n*8+c] += X[p, n, 8*j+c]/8192   (identity-matmul partial
            sums accumulated in one PSUM bank; W/8 matmuls on the idle PE)
      DVE:  S[p,n] = reduce_add(PS view)      (fp32, only nb*8 elements)
      DVE:  R16d[p,n,t] = recip_approx_fast(S[p,n]) -> f16, 8-dup
            (one custom-DVE op: broadcast read + f16 write fuse the
            reciprocal, the f16 cast and the 8x duplication)
      DVE:  A = X * R16d                      (f16 2x mode: the stride-0
            broadcast hides in a middle dim of a [P,nb,W/8,8] view while
            the innermost dim reads unit-stride 8-duplicated runs)
  - Chunk processing is big-first after the small opener: chunk 0 is small
    so the first exp starts during the DMA latency, then the largest chunks
    run while the DVE backlog can still hide behind the exp stream, and the
    tail chunks are tiny so the final store chain drains quickly.  The
    final store issues on the ACT HWDGE ring (idle after the last exp) so
    it does not queue behind the penultimate store on the Sync ring.
  - Host divides by 8192 (exact) while scattering back to original order.
  All arithmetic (exp, segment sums, reciprocal, normalize) runs on device;
  the host only sorts/pads/permutes layouts.
"""
import sys

sys.path.insert(0, "/opt/trn_rl_repo")

import numpy as np
import concourse.bacc as bacc
import concourse.mybir as mybir
import concourse.tile as tile
from concourse import bass_utils
from concourse.dve_ops import RECIP_APPROX_FAST_CONSTS, RECIPROCAL_APPROX_FAST

P = 128
N_CORES = 8
NUM_EDGES = 6_400_000
NUM_NODES = 100_000
NPC = NUM_NODES // N_CORES          # 12500 nodes per core
NBLK = (NPC + P - 1) // P           # 98 node blocks per core
SCALE = 8192.0                      # keeps alpha*SCALE in fp16 normal range
FIRST_FD = 512                      # small first chunk: start compute early
TARGET_FD = 1792                    # mid chunks
TAIL_FDS = (640, 320)               # taper the tail: drain quickly
CAP_FD = 2304
MAXNB = 64
MM_W = 8                            # identity-matmul sub-slice width
DUP = 8                             # duplication factor of the R row

f16, f32 = mybir.dt.float16, mybir.dt.float32
_cache = {}


def make_plan(counts):
    """Chunk plan [(nb, W), ...] covering the NBLK degree-sorted blocks."""
    deg_sorted = np.sort(counts.reshape(N_CORES, NPC), axis=1)
    pad = NBLK * P - NPC
    deg_sorted = np.pad(deg_sorted, ((0, 0), (0, pad)))
    blockmax = deg_sorted.reshape(N_CORES, NBLK, P).max(axis=2).max(axis=0)
    blockmax = np.maximum(blockmax, 1)
    wof = ((blockmax + 7) // 8) * 8          # per-block width if chunk ended
    plan = []
    b = 0
    while b < NBLK:
        ci = len(plan)
        tgt = FIRST_FD if ci == 0 else (1024 if ci == 1 else TARGET_FD)
        nb = 1
        while b + nb < NBLK and (nb + 1) * wof[b + nb] <= tgt:
            nb += 1
        plan.append((int(nb), int(wof[b + nb - 1])))
        b += nb
    # carve small tail chunks so the final stores are short
    for tail_fd in TAIL_FDS:
        if len(plan) > 1 and plan[-1][0] * plan[-1][1] > tail_fd:
            nb, W = plan.pop()
            nb_tail = max(1, tail_fd // W)
            if nb > nb_tail:
                plan.append((nb - nb_tail, W))
            plan.append((min(nb, nb_tail), W))
    return tuple(plan)


def plan_layout(plan):
    """Per-block (colbase, width) arrays and chunk offsets.

    Column layout: [identity (P cols) | chunk 0 | chunk 1 | ...]."""
    W_blk = np.empty(NBLK, dtype=np.int64)
    base_blk = np.empty(NBLK, dtype=np.int64)
    chunk_off = []
    o = P
    b = 0
    for nb, W in plan:
        chunk_off.append(o)
        for i in range(nb):
            W_blk[b + i] = W
            base_blk[b + i] = o + i * W
        o += nb * W
        b += nb
    assert b == NBLK
    return W_blk, base_blk, chunk_off, o


PSUM_BANK = 512                     # one PSUM bank: 512 fp32 per partition


def process_order(plan):
    """Natural plan order: ramps 512 -> 1024 -> big mids -> tapered tail.
    With per-chunk latency halved by the nb-split, the tail chunks' tiny
    chains finish right after their (late-arriving) loads."""
    return list(range(len(plan)))


def _build(plan):
    W_blk, base_blk, chunk_off, FD = plan_layout(plan)
    nc = bacc.Bacc("TRN2", target_bir_lowering=False, debug=False,
                   enable_asserts=False)
    d_E = nc.dram_tensor("E", [P, FD], f16, kind="ExternalInput")
    d_A = nc.dram_tensor("alpha", [P, FD], f16, kind="ExternalOutput")
    OP = mybir.AluOpType
    Exp = mybir.ActivationFunctionType.Exp
    order = process_order(plan)
    rc = RECIP_APPROX_FAST_CONSTS

    with tile.TileContext(nc) as tc:
        with (
            tc.tile_pool(name="const", bufs=1) as cpool,
            tc.tile_pool(name="io", bufs=6) as iopool,
            tc.tile_pool(name="sm", bufs=4) as spool,
            tc.tile_pool(name="ps", bufs=3, space="PSUM") as ppool,
        ):
            # first DMA: the scaled identity + the small chunk 0, so exp and
            # the first PE matmuls start as soon as one small load lands
            c0_end = chunk_off[0] + plan[0][0] * plan[0][1]
            E0 = cpool.tile([P, c0_end], f16)
            nc.sync.dma_start(out=E0[:], in_=d_E[:, 0:c0_end])
            ident = E0[:, 0:P]
            for ci in order:
                nb, W = plan[ci]
                o_lo = chunk_off[ci]
                fdc = nb * W
                assert nb <= MAXNB and fdc <= CAP_FD
                assert W % MM_W == 0 and nb * MM_W <= PSUM_BANK
                if ci == 0:
                    E16v = E0[:, o_lo:o_lo + fdc]
                else:
                    E16 = iopool.tile([P, CAP_FD], f16, tag="E16")
                    nc.sync.dma_start(out=E16[:, 0:fdc],
                                      in_=d_E[:, o_lo:o_lo + fdc])
                    E16v = E16[:, 0:fdc]
                X16 = iopool.tile([P, CAP_FD], f16, tag="X16")
                A16 = iopool.tile([P, CAP_FD], f16, tag="A16")
                # split big chunks' compute into two nb-halves: half 1's
                # exp/PE/reduce/recip/multiply/store all run while half 2 is
                # still in exp, so stores start ~2us earlier per chunk and
                # the DMA wire never goes idle between the load stream and
                # the store stream
                gW = W // DUP
                subs = ([0, (nb + 1) // 2, nb] if fdc >= 512 else [0, nb])
                for lo, hi in zip(subs, subs[1:]):
                    nbs = hi - lo
                    c_lo = lo * W
                    c_hi = hi * W
                    nc.scalar.activation(X16[:, c_lo:c_hi],
                                         E16v[:, c_lo:c_hi], Exp)
                    # segment partial sums on the (otherwise idle) PE:
                    # accumulate identity-matmuls of MM_W-wide sub-slices
                    # into PSUM, so the vector engine only reduces nb*MM_W
                    v = X16[:, c_lo:c_hi].rearrange("p (n d) -> p n d", d=W)
                    S = spool.tile([P, MAXNB], f32, tag="S")
                    PS = ppool.tile([P, PSUM_BANK], f32, space="PSUM",
                                    tag="PS")
                    nmm = W // MM_W
                    for j in range(nmm):
                        nc.tensor.matmul(out=PS[:, 0:nbs * MM_W],
                                         lhsT=ident[:],
                                         rhs=v[:, :, MM_W * j:MM_W * (j + 1)],
                                         start=(j == 0), stop=(j == nmm - 1))
                    nc.vector.tensor_reduce(
                        out=S[:, 0:nbs],
                        in_=PS[:, 0:nbs * MM_W].rearrange("p (n d) -> p n d",
                                                          d=MM_W),
                        axis=mybir.AxisListType.X, op=OP.add)
                    # one fused custom-DVE op: R16d[p, n, t] = f16(1/S[p, n])
                    # (= 8192/sum since the identity is pre-scaled), written
                    # as 8-duplicated unit-stride runs for the broadcast
                    # multiply
                    R16d = spool.tile([P, DUP * MAXNB], f16, tag="R16d")
                    nc.vector._custom_dve(
                        RECIPROCAL_APPROX_FAST,
                        out=R16d[:, 0:DUP * nbs].rearrange(
                            "p (n t) -> p n t", t=DUP),
                        in0=S[:, 0:nbs].unsqueeze(2).broadcast_to(
                            [P, nbs, DUP]),
                        s0=rc["s0"], s1=rc["s1"], imm2=rc["imm2"])
                    xv = X16[:, c_lo:c_hi].rearrange("p (n g t) -> p n g t",
                                                     g=gW, t=DUP)
                    av = A16[:, c_lo:c_hi].rearrange("p (n g t) -> p n g t",
                                                     g=gW, t=DUP)
                    rb = R16d[:, 0:DUP * nbs].rearrange(
                        "p (n t) -> p n t", t=DUP).unsqueeze(2).broadcast_to(
                        [P, nbs, gW, DUP])
                    nc.vector.tensor_tensor(out=av, in0=xv, in1=rb,
                                            op=OP.mult)
                    # the final store issues on the ACT HWDGE ring (idle once
                    # the last exp is done) so it drains in parallel with the
                    # penultimate store on the Sync ring instead of behind it
                    store_eng = (nc.scalar
                                 if ci == order[-1] and hi == nb else nc.sync)
                    store_eng.dma_start(
                        out=d_A[:, o_lo + c_lo:o_lo + c_hi],
                        in_=A16[:, c_lo:c_hi])
    nc.compile()
    return nc


def _get_neff(plan):
    if plan not in _cache:
        _cache[plan] = _build(plan)
    return _cache[plan]


def prep_inputs(e, edge_index):
    """Sort edges by target node, degree-sort nodes, chunk-width padding."""
    e = np.asarray(e, dtype=np.float32).reshape(-1)
    t = np.asarray(edge_index)[1].astype(np.int64)
    counts = np.bincount(t, minlength=NUM_NODES)
    plan = make_plan(counts)
    W_blk, base_blk, chunk_off, FD = plan_layout(plan)
    # node -> rank within its core under ascending-degree order
    order = np.argsort(counts.reshape(N_CORES, NPC), axis=1, kind="stable")
    m_of = np.empty((N_CORES, NPC), dtype=np.int64)
    ar = np.arange(NPC, dtype=np.int64)
    for c in range(N_CORES):
        m_of[c, order[c]] = ar
    m = m_of.reshape(-1)                    # global node -> rank in core
    p_of = m % P
    colbase = base_blk[m // P]              # start column per node
    # per-edge destination in the padded layout
    perm = np.argsort(t, kind="stable")
    t_s = t[perm]
    starts = np.zeros(NUM_NODES + 1, dtype=np.int64)
    np.cumsum(counts, out=starts[1:])
    rank = np.arange(NUM_EDGES, dtype=np.int64) - starts[t_s]
    c_e = t_s // NPC
    flat = (c_e * P + p_of[t_s]) * FD + colbase[t_s] + rank
    E = np.full(N_CORES * P * FD, -100.0, dtype=np.float16)
    E[flat] = e[perm].astype(np.float16)
    E = E.reshape(N_CORES, P, FD)
    # scaled identity at column 0: PE sums come out as S/8192, so the
    # reciprocal directly yields 8192/S (1/8192 is exact in fp16)
    E[:, :, 0:P] = (np.eye(P, dtype=np.float16) / np.float16(SCALE))[None]
    return E, flat, perm, plan


def make_in_maps(E):
    return [{"E": E[c]} for c in range(N_CORES)]


def kernel(e, edge_index, num_nodes):
    assert int(num_nodes) == NUM_NODES
    E, flat, perm, plan = prep_inputs(e, edge_index)
    nc = _get_neff(plan)
    in_maps = make_in_maps(E)
    res = bass_utils.run_bass_kernel_spmd(nc, in_maps,
                                          core_ids=list(range(N_CORES)))
    A = np.stack([np.asarray(res.results[c]["alpha"])
                  for c in range(N_CORES)])
    alpha_sorted = A.reshape(-1)[flat].astype(np.float32) * np.float32(1.0 / SCALE)
    out = np.empty(NUM_EDGES, dtype=np.float32)
    out[perm] = alpha_sorted
    return out
